# revision 49
# baseline (speedup 1.0000x reference)
"""GINE layer (gather + edge-linear + scatter-mean + node MLP + BatchNorm + ReLU)
as a distributed Bass kernel on 8 TRN2 NeuronCores.

Sharding: edges are sharded by destination-node slab (N/8 nodes per core), so
each core's scatter-sums are complete locally.  Full x (bf16) is replicated in
each core's DRAM as the gather table; only BatchNorm statistics are
all-reduced.

Gather-descriptor pairing: random 256B-row gathers are descriptor-bound, so
edges that share a source node are paired into one 512B descriptor read from a
row-duplicated table x2 = [x|x].  A pair lands in two 128-slot slices of one
column; each slice is destination-block pure, so the scatter stays the plain
one-hot matmul.  Pairs are limited to block gap <= WG so at most WG+1
partial-sum blocks are open in PSUM at once.  The schedule (columns per
(half, b0, gap) class, single chunks per (half, block)) is sized by the max
over cores so the SPMD graph is shared; per-core slack is filled with
singleton edges, then sentinel pads.

Device pipeline per core:
  phase 1: dma_gather pair columns (512B descs) + single chunks (256B descs),
           build one-hot dst matrices on DVE (tensor_scalar is_equal vs iota,
           4x perf mode), TensorE matmul-scatter into per-block PSUM
           accumulators (window of WG+2 open blocks); per-block epilogue uses
           host-side cnt/attr-sum tables, PE-transposes into h_T.
  phase 2 (interleaved): node MLP on 512-node strips + BN stat accumulation.
  tail:    AllGather [sum, sumsq], normalize + relu, DMA out (bf16).
"""

import sys

sys.path.insert(0, "/opt/trn_rl_repo")

import numpy as np
import ml_dtypes

import concourse.bacc as bacc
import concourse.bass as bass
from concourse import mybir
from concourse.tile import TileContext
from concourse import bass_utils

BF16 = ml_dtypes.bfloat16

N = 50000
E = 1600000
C = 128
NCORES = 8
NSLAB = N // NCORES          # 6250 nodes per core
NBLK = (NSLAB + 127) // 128  # 49 dst blocks per core
HALF = N // 2                # 25000 (int16 gather index limit is 32767)
SENTINEL = 200.0             # never matches iota 0..127
BN_EPS = 1e-5
WG = 6                       # max block gap within a pair

# knobs (settable by test harness)
TRACE = False
LAST_EXEC_NS = None
LAST_RESULTS = None
COLLECTIVE = True
RACE_DETECT = True


def _preprocess(x, edge_index, edge_attr):
    """Host-side sharding + pairing. Returns (schedule, per-core maps)."""
    src = np.asarray(edge_index[0], dtype=np.int64)
    dst = np.asarray(edge_index[1], dtype=np.int64)
    attr = np.asarray(edge_attr[:, 0], dtype=np.float32)

    core_of = dst // NSLAB
    NCLS = 2 * NBLK * (WG + 1)          # class id = (h*NBLK + b0)*(WG+1) + g

    percore = []
    for i in range(NCORES):
        m = core_of == i
        s, d, a = src[m], dst[m] - i * NSLAB, attr[m]
        b = d // 128
        order = np.lexsort((b, s))
        s, d, b, a = s[order], d[order], b[order], a[order]
        Ei = len(s)
        # host-side scatter-mean denominators
        deg = np.bincount(d, minlength=NSLAB).astype(np.float32)
        asum = np.bincount(d, weights=a, minlength=NSLAB).astype(np.float32)
        # same-source adjacent pairing with block gap <= WG
        cand = np.zeros(Ei, bool)
        cand[:-1] = (s[1:] == s[:-1]) & ((b[1:] - b[:-1]) <= WG)
        startrun = cand & ~np.r_[False, cand[:-1]]
        runstart = np.maximum.accumulate(
            np.where(startrun, np.arange(Ei), 0))
        p0 = cand & ((np.arange(Ei) - runstart) % 2 == 0)
        p1 = np.r_[False, p0[:-1]]
        single = ~(p0 | p1)
        h = (s >= HALF).astype(np.int64)
        srel = s - h * HALF
        i1 = np.where(p0)[0] + 1
        cls = (h[p0] * NBLK + b[p0]) * (WG + 1) + (b[i1] - b[p0])
        pair_rec = (cls, srel[p0], (d[p0] % 128).astype(np.float32),
                    (d[i1] % 128).astype(np.float32))
        skey = h[single] * NBLK + b[single]
        sing_rec = (skey, srel[single], (d[single] % 128).astype(np.float32))
        percore.append((pair_rec, sing_rec, deg, asum))

    npairs = np.zeros((NCORES, NCLS), np.int64)
    for i in range(NCORES):
        npairs[i] = np.bincount(percore[i][0][0], minlength=NCLS)
    paircols = (npairs.max(axis=0) + 127) // 128

    # per-core leftover singles after filling pair-column slack
    nsing_left = np.zeros((NCORES, 2 * NBLK), np.int64)
    for i in range(NCORES):
        skey = percore[i][1][0]
        savail = np.bincount(skey, minlength=2 * NBLK)
        for c in np.where(paircols > 0)[0]:
            hb0, g = divmod(c, WG + 1)
            slack = paircols[c] * 128 - npairs[i][c]
            for k in (hb0, hb0 + g):
                take = min(slack, savail[k])
                savail[k] -= take
                slack -= take
        nsing_left[i] = savail
    singlechunks = (nsing_left.max(axis=0) + 127) // 128

    # shared emission order
    units = []          # ('P', h, b0, g) / ('S', h, b)
    for b in range(NBLK):
        for g in range(WG + 1):
            for h in (0, 1):
                c = (h * NBLK + b) * (WG + 1) + g
                if b + g < NBLK and paircols[c]:
                    units += [("P", h, b, g)] * int(paircols[c])
        for h in (0, 1):
            k = h * NBLK + b
            if singlechunks[k]:
                units += [("S", h, b)] * int(singlechunks[k])
    NSLICE = sum(2 if u[0] == "P" else 1 for u in units)
    pcols_h = [sum(1 for u in units if u[0] == "P" and u[1] == h)
               for h in (0, 1)]
    schk_h = [sum(1 for u in units if u[0] == "S" and u[1] == h)
              for h in (0, 1)]
    schedule = (tuple(units), tuple(pcols_h), tuple(schk_h), NSLICE)

    # per-core fill
    maps = []
    for i in range(NCORES):
        (pcls, psrel, pd0, pd1), (skey, ssrel, sdrel), deg, asum = percore[i]
        pair_by_cls = {}
        o = np.argsort(pcls, kind="stable")
        for c, lo, hi in zip(*_runs(pcls[o])):
            pair_by_cls[c] = (psrel[o[lo:hi]], pd0[o[lo:hi]], pd1[o[lo:hi]])
        sing_by_key = {}
        o = np.argsort(skey, kind="stable")
        for k, lo, hi in zip(*_runs(skey[o])):
            sing_by_key[k] = [ssrel[o[lo:hi]], sdrel[o[lo:hi]], 0]

        pidx = [np.zeros(pcols_h[h] * 128, np.int16) for h in (0, 1)]
        sidx = [np.zeros(schk_h[h] * 128, np.int16) for h in (0, 1)]
        dstrel = np.full((NSLICE, 128), SENTINEL, np.float32)
        pcur, scur, slice_i = [0, 0], [0, 0], 0
        ccur = {}
        for u in units:
            if u[0] == "P":
                h, b0, g = u[1], u[2], u[3]
                c = (h * NBLK + b0) * (WG + 1) + g
                pool = pair_by_cls.get(c)
                lo = ccur.get(c, 0)
                take = min(128, (len(pool[0]) if pool else 0) - lo)
                take = max(take, 0)
                ccur[c] = lo + take
                base = pcur[h] * 128
                if take:
                    pidx[h][base:base + take] = pool[0][lo:lo + take]
                    dstrel[slice_i, :take] = pool[1][lo:lo + take]
                    dstrel[slice_i + 1, :take] = pool[2][lo:lo + take]
                fill = 128 - take
                pos = take
                for sl, kk in ((0, h * NBLK + b0), (1, h * NBLK + b0 + g)):
                    if fill <= 0:
                        break
                    q = sing_by_key.get(kk)
                    if q is None:
                        continue
                    av = len(q[0]) - q[2]
                    t = min(fill, av)
                    if t > 0:
                        qlo = q[2]
                        pidx[h][base + pos:base + pos + t] = q[0][qlo:qlo + t]
                        dstrel[slice_i + sl, pos:pos + t] = q[1][qlo:qlo + t]
                        q[2] += t
                        pos += t
                        fill -= t
                pcur[h] += 1
                slice_i += 2
            else:
                h, b = u[1], u[2]
                q = sing_by_key.get(h * NBLK + b)
                av = (len(q[0]) - q[2]) if q else 0
                take = min(128, max(av, 0))
                base = scur[h] * 128
                if take:
                    qlo = q[2]
                    sidx[h][base:base + take] = q[0][qlo:qlo + take]
                    dstrel[slice_i, :take] = q[1][qlo:qlo + take]
                    q[2] += take
                scur[h] += 1
                slice_i += 1

        def wrap(v):
            # dma_gather idx layout: [128, n//16] wrapped in 16 partitions,
            # replicated for the 8 Q7 cores
            if len(v) == 0:
                return np.zeros((128, 0), np.int16)
            return np.ascontiguousarray(np.tile(v.reshape(-1, 16).T, (8, 1)))

        cna = np.zeros((128, 2 * NBLK), dtype=np.float32)
        padded = np.zeros(NBLK * 128, dtype=np.float32)
        padded[:NSLAB] = deg
        cna[:, 0:NBLK] = padded.reshape(NBLK, 128).T
        padded = np.zeros(NBLK * 128, dtype=np.float32)
        padded[:NSLAB] = asum
        cna[:, NBLK:2 * NBLK] = padded.reshape(NBLK, 128).T
        maps.append({
            "pidx0": wrap(pidx[0]), "pidx1": wrap(pidx[1]),
            "sidx0": wrap(sidx[0]), "sidx1": wrap(sidx[1]),
            "dstrel": np.ascontiguousarray(dstrel.T),
            "cna": cna,
        })
    return schedule, maps


def _runs(sorted_vals):
    """(values, run_starts, run_ends) for a sorted 1-D array."""
    if len(sorted_vals) == 0:
        return [], [], []
    change = np.r_[True, sorted_vals[1:] != sorted_vals[:-1]]
    starts = np.where(change)[0]
    ends = np.r_[starts[1:], len(sorted_vals)]
    return sorted_vals[starts], starts, ends


def _build_graph(schedule, eps1):
    """Build the SPMD Bass graph (same for all cores)."""
    f32 = mybir.dt.float32
    bf16 = mybir.dt.bfloat16
    units, pcols_h, schk_h, NSLICE = schedule
    G_CH = 8           # chunks/columns per gather call (1024-desc ring limit)
    NSTRIP = NBLK      # one MLP strip per 128-node dst block

    nc = bacc.Bacc("TRN2", num_devices=NCORES, detect_race_conditions=RACE_DETECT)

    x_d = nc.declare_dram_parameter("x_bf16", [N, C], bf16, isOutput=False)
    x2_d = nc.declare_dram_parameter("x2_bf16", [N, 2 * C], bf16, isOutput=False)
    pidx_d = [nc.declare_dram_parameter(f"pidx{h}", [128, max(pcols_h[h], 1) * 8],
                                        mybir.dt.int16, isOutput=False)
              for h in (0, 1)]
    sidx_d = [nc.declare_dram_parameter(f"sidx{h}", [128, max(schk_h[h], 1) * 8],
                                        mybir.dt.int16, isOutput=False)
              for h in (0, 1)]
    dstrel_d = nc.declare_dram_parameter("dstrel", [128, NSLICE], f32, isOutput=False)
    cna_d = nc.declare_dram_parameter("cna", [128, 2 * NBLK], f32, isOutput=False)
    xt_d = nc.declare_dram_parameter("x_t", [128, NSLAB], bf16, isOutput=False)
    cf_d = nc.declare_dram_parameter("consts_f32", [128, 389], f32, isOutput=False)
    iob_d = nc.declare_dram_parameter("iota_ones", [128, 257], bf16, isOutput=False)
    wts_d = nc.declare_dram_parameter("wts", [128, 384], bf16, isOutput=False)
    out_d = nc.declare_dram_parameter("out", [128, NSLAB], bf16, isOutput=True)

    bn_in_d = nc.dram_tensor("bn_in", [128, 2], f32, kind="Internal")
    bn_out_d = nc.dram_tensor("bn_out", [NCORES * 128, 2], f32, kind="Internal", addr_space="Shared")

    xh = (x_d[0:HALF, :], x_d[HALF:N, :])
    x2h = (x2_d[0:HALF, :], x2_d[HALF:N, :])

    # per-block total touches (slices) for start/stop flags
    touches = [0] * NBLK
    for u in units:
        if u[0] == "P":
            touches[u[2]] += 1
            touches[u[2] + u[3]] += 1
        else:
            touches[u[2]] += 1



    with TileContext(nc) as tc:
        with tc.tile_pool(name="persist", bufs=1) as pp, \
             tc.tile_pool(name="gpair", bufs=6) as gpp, \
             tc.tile_pool(name="gsing", bufs=6) as gsp, \
             tc.tile_pool(name="spool", bufs=6) as sp, \
             tc.tile_pool(name="eppool", bufs=2) as ep, \
             tc.tile_pool(name="p2pool", bufs=3) as p2, \
             tc.tile_pool(name="p1psum", bufs=WG + 1, space="PSUM") as p1p, \
             tc.tile_pool(name="auxpsum", bufs=1, space="PSUM") as auxp:
            pidx_sb = [pp.tile([128, max(pcols_h[h], 1) * 8], mybir.dt.int16,
                               name=f"pidx_sb{h}") for h in (0, 1)]
            sidx_sb = [pp.tile([128, max(schk_h[h], 1) * 8], mybir.dt.int16,
                               name=f"sidx_sb{h}") for h in (0, 1)]
            dstrel_sb = pp.tile([128, NSLICE], f32)
            cna_sb = pp.tile([128, 2 * NBLK], f32)
            xt_sb = pp.tile([128, NSLAB], bf16)
            cf_sb = pp.tile([128, 389], f32)
            iob_sb = pp.tile([128, 257], bf16)
            wts_sb = pp.tile([128, 384], bf16)
            ht_sb = pp.tile([128, NSLAB], bf16)
            opre_sb = pp.tile([128, NSLAB], bf16)

            for h in (0, 1):
                w = max(pcols_h[h], 1) * 8
                nc.sync.dma_start(out=pidx_sb[h][:, 0:min(128, w)],
                                  in_=pidx_d[h][:, 0:min(128, w)])
                if w > 128:
                    nc.sync.dma_start(out=pidx_sb[h][:, 128:w],
                                      in_=pidx_d[h][:, 128:w])
                nc.sync.dma_start(out=sidx_sb[h][:], in_=sidx_d[h][:])
            # ordered by first use: iota gates the first one-hot build,
            # dstrel the first matmuls, cf the first epilogue
            nc.scalar.dma_start(out=iob_sb[:], in_=iob_d[:])
            nc.scalar.dma_start(out=dstrel_sb[:, 0:64], in_=dstrel_d[:, 0:64])
            nc.scalar.dma_start(out=dstrel_sb[:, 64:NSLICE],
                                in_=dstrel_d[:, 64:NSLICE])
            nc.scalar.dma_start(out=cf_sb[:], in_=cf_d[:])
            nc.scalar.dma_start(out=cna_sb[:], in_=cna_d[:])
            nc.scalar.dma_start(out=xt_sb[:], in_=xt_d[:])
            nc.scalar.dma_start(out=wts_sb[:], in_=wts_d[:])

            ew_b = cf_sb[:, 0:128]
            eb_b = cf_sb[:, 128:256]
            ident = cf_sb[:, 256:384]
            b1_c = cf_sb[:, 384:385]
            b2pr_c = cf_sb[:, 385:386]
            gamma_c = cf_sb[:, 386:387]
            beta_c = cf_sb[:, 387:388]
            bneps_c = cf_sb[:, 388:389]
            iota128 = iob_sb[:, 0:128]
            identeps_bf = iob_sb[:, 129:257]
            w1_s = wts_sb[:, 0:128]
            w2_s = wts_sb[:, 128:256]
            rw_s = wts_sb[:, 256:384]

            sum_cols = p2.tile([128, NSTRIP], f32, tag="sumc")
            sq_cols = p2.tile([128, NSTRIP], f32, tag="sqc")

            # touch Sqrt before the busy phase so the ACT function-table
            # load (1.3us) doesn't land on the BN tail's critical path
            warm = p2.tile([128, 1], f32, tag="warm")
            nc.scalar.activation(out=warm[:], in_=cf_sb[:, 388:389],
                                 func=mybir.ActivationFunctionType.Sqrt,
                                 bias=0.0, scale=1.0)

            def psum_slice(b):
                if b not in open_psum:
                    open_psum[b] = p1p.tile([128, C], f32, tag="scat",
                                            name=f"scat{b}")
                return open_psum[b][:]

            def epilogue(b):
                ncol = NSLAB - b * 128 if b == NBLK - 1 else 128
                pt = psum_slice(b)
                blk = ep.tile([128, C], f32, tag="blk")
                nc.scalar.copy(out=blk[:], in_=pt[:])
                cnt_c = cna_sb[:, b:b + 1]
                asum_c = cna_sb[:, NBLK + b:NBLK + b + 1]
                cmax = ep.tile([128, 1], f32, tag="cmax")
                nc.vector.tensor_scalar_max(
                    out=cmax[:], in0=cnt_c, scalar1=1.0)
                recip = ep.tile([128, 1], f32, tag="recip")
                nc.vector.reciprocal(recip[:], cmax[:])
                t1 = ep.tile([128, C], f32, tag="ep1")
                nc.vector.scalar_tensor_tensor(
                    out=t1[:], in0=ew_b, scalar=asum_c,
                    in1=blk[:, 0:C],
                    op0=mybir.AluOpType.mult, op1=mybir.AluOpType.add)
                nc.vector.scalar_tensor_tensor(
                    out=t1[:], in0=eb_b, scalar=cnt_c,
                    in1=t1[:],
                    op0=mybir.AluOpType.mult, op1=mybir.AluOpType.add)
                aggr = ep.tile([128, C], f32, tag="aggr")
                nc.scalar.mul(out=aggr[:], in_=t1[:], mul=recip[:])
                # transpose (tp), hidden (pa) and output (po) partial sums all
                # share one PSUM bank; their accumulation groups are strictly
                # sequential (each is fully consumed before the next starts)
                aux = auxp.tile([128, 384], f32, tag="aux", name=f"aux{b}")
                ptt = aux[:, 0:128]
                pa = aux[:, 128:256]
                po = aux[:, 256:384]
                n0 = b * 128
                nc.tensor.matmul(out=ptt, lhsT=aggr[:], rhs=ident,
                                 is_transpose=True, start=True, stop=False)
                # accumulate (1+eps)*x_T via (eps1*I).T @ x_T on PE
                nc.tensor.matmul(out=ptt[:, 0:ncol], lhsT=identeps_bf,
                                 rhs=xt_sb[:, n0:n0 + ncol],
                                 start=False, stop=True)
                nc.scalar.copy(out=ht_sb[:, n0:n0 + ncol], in_=ptt[:, 0:ncol])
                # node-MLP strip for this block (strip index == block index)
                w = ncol
                nc.tensor.matmul(out=pa[:, :w], lhsT=w1_s,
                                 rhs=ht_sb[:, n0:n0 + w], start=True, stop=True)
                hid = p2.tile([128, 128], bf16, tag="hid")
                nc.scalar.activation(out=hid[:, :w], in_=pa[:, :w],
                                     func=mybir.ActivationFunctionType.Relu,
                                     bias=b1_c, scale=1.0)
                nc.tensor.matmul(out=po[:, :w], lhsT=w2_s, rhs=hid[:, :w],
                                 start=True, stop=False)
                nc.tensor.matmul(out=po[:, :w], lhsT=rw_s,
                                 rhs=xt_sb[:, n0:n0 + w], start=False, stop=True)
                nc.scalar.activation(out=opre_sb[:, n0:n0 + w], in_=po[:, :w],
                                     func=mybir.ActivationFunctionType.Identity,
                                     bias=b2pr_c, scale=1.0,
                                     accum_out=sum_cols[:, b:b + 1])
                sq = p2.tile([128, 128], f32, tag="sq")
                nc.scalar.activation(out=sq[:, :w], in_=opre_sb[:, n0:n0 + w],
                                     func=mybir.ActivationFunctionType.Square,
                                     accum_out=sq_cols[:, b:b + 1])

            # ---------------- phase 1 ----------------
            # 4 lazy gather streams: (pair, half) and (single, half)
            cursors = {("P", 0): 0, ("P", 1): 0, ("S", 0): 0, ("S", 1): 0}
            tiles = {k: [] for k in cursors}
            totals = {("P", 0): pcols_h[0], ("P", 1): pcols_h[1],
                      ("S", 0): schk_h[0], ("S", 1): schk_h[1]}

            def ensure(kind, h, col):
                # issue eagerly ~2 calls ahead of consumption so the Q7
                # descriptor generation never gates the SDMA stream
                key = (kind, h)
                while (not tiles[key] or tiles[key][-1][1] <= col
                       or (tiles[key][-1][1] < min(col + 2 * G_CH, totals[key])
                           and len(tiles[key]) < 3)):
                    lo = tiles[key][-1][1] if tiles[key] else 0
                    g = min(G_CH, totals[key] - lo)
                    if kind == "P":
                        gt = gpp.tile([128, g, 2 * C], bf16, tag="gpa")
                        nc.gpsimd.dma_gather(
                            out_ap=gt[:], in_ap=x2h[h],
                            idxs_ap=pidx_sb[h][:, lo * 8:(lo + g) * 8],
                            num_idxs=g * 128, num_idxs_reg=g * 128,
                            elem_size=2 * C)
                    else:
                        gt = gsp.tile([128, g, C], bf16, tag="gsi")
                        nc.gpsimd.dma_gather(
                            out_ap=gt[:], in_ap=xh[h],
                            idxs_ap=sidx_sb[h][:, lo * 8:(lo + g) * 8],
                            num_idxs=g * 128, num_idxs_reg=g * 128,
                            elem_size=C)
                    tiles[key].append((lo, lo + g, gt))
                    if len(tiles[key]) > 3:
                        tiles[key].pop(0)
                for lo, hi, gt in tiles[key]:
                    if lo <= col < hi:
                        return gt, col - lo
                raise AssertionError("gather tile evicted too early")

            open_psum = {}
            seen = [0] * NBLK
            stream_pos = {k: 0 for k in cursors}
            slice_i = 0

            def scat(bj, stile, rhs):
                seen[bj] += 1
                nc.tensor.matmul(out=psum_slice(bj), lhsT=stile, rhs=rhs,
                                 start=seen[bj] == 1,
                                 stop=seen[bj] == touches[bj])
                if seen[bj] == touches[bj]:
                    epilogue(bj)

            for u in units:
                if u[0] == "P":
                    h, b0, g = u[1], u[2], u[3]
                    col = stream_pos[("P", h)]
                    stream_pos[("P", h)] += 1
                    gt, lc = ensure("P", h, col)
                    for sl, bj in ((0, b0), (1, b0 + g)):
                        stile = sp.tile([128, 128], bf16, tag="sel")
                        nc.vector.tensor_scalar(
                            out=stile[:], in0=iota128,
                            scalar1=dstrel_sb[:, slice_i:slice_i + 1],
                            scalar2=None, op0=mybir.AluOpType.is_equal)
                        scat(bj, stile[:], gt[:, lc, sl * C:(sl + 1) * C])
                        slice_i += 1
                else:
                    h, b = u[1], u[2]
                    col = stream_pos[("S", h)]
                    stream_pos[("S", h)] += 1
                    gt, lc = ensure("S", h, col)
                    stile = sp.tile([128, 128], bf16, tag="sel")
                    nc.vector.tensor_scalar(
                        out=stile[:], in0=iota128,
                        scalar1=dstrel_sb[:, slice_i:slice_i + 1],
                        scalar2=None, op0=mybir.AluOpType.is_equal)
                    scat(b, stile[:], gt[:, lc, :])
                    slice_i += 1

            # blocks with zero touches (possible if a block got no edges)
            for b in range(NBLK):
                if touches[b] == 0:
                    zt = sp.tile([128, 128], bf16, tag="sel")
                    nc.vector.tensor_scalar(
                        out=zt[:], in0=iota128, scalar1=-1.0, scalar2=None,
                        op0=mybir.AluOpType.is_equal)
                    nc.tensor.matmul(out=psum_slice(b), lhsT=zt[:],
                                     rhs=xt_sb[:, 0:C], start=True, stop=True)
                    epilogue(b)

            # ---------------- BN tail ----------------
            ssum = p2.tile([128, 1], f32, tag="ssum")
            ssq = p2.tile([128, 1], f32, tag="ssq")
            nc.vector.tensor_reduce(out=ssum[:], in_=sum_cols[:],
                                    axis=mybir.AxisListType.X,
                                    op=mybir.AluOpType.add)
            nc.vector.tensor_reduce(out=ssq[:], in_=sq_cols[:],
                                    axis=mybir.AxisListType.X,
                                    op=mybir.AluOpType.add)
            bn_sb = p2.tile([128, 2], f32, tag="bn")
            nc.vector.tensor_copy(out=bn_sb[:, 0:1], in_=ssum[:])
            nc.vector.tensor_copy(out=bn_sb[:, 1:2], in_=ssq[:])
            nc.sync.dma_start(out=bn_in_d[:], in_=bn_sb[:])
            bn2 = p2.tile([128, 2], f32, tag="bn2")
            if COLLECTIVE:
                # AllGather + local reduce: priced well below AllReduce for
                # tiny payloads
                nc.gpsimd.collective_compute(
                    "AllGather", mybir.AluOpType.bypass,
                    replica_groups=[list(range(NCORES))],
                    ins=[bn_in_d[:].opt()], outs=[bn_out_d[:].opt()])
                bn8 = p2.tile([128, NCORES, 2], f32, tag="bn8")
                nc.sync.dma_start(
                    out=bn8[:],
                    in_=bass.AP(bn_out_d, 0, [(2, 128), (256, NCORES), (1, 2)]))
                bn8r = bn8[:]
                bn8v = bass.AP(bn8r.tensor, bn8r.offset,
                               [bn8r.ap[0], (1, 2), (2, NCORES)])
                nc.vector.tensor_reduce(out=bn2[:], in_=bn8v,
                                        axis=mybir.AxisListType.X,
                                        op=mybir.AluOpType.add)
                nmean = N
            else:
                nc.sync.dma_start(out=bn2[:], in_=bn_in_d[:])
                nmean = NSLAB

            mean = p2.tile([128, 1], f32, tag="mean")
            ex2 = p2.tile([128, 1], f32, tag="ex2")
            nc.scalar.mul(out=mean[:], in_=bn2[:, 0:1], mul=1.0 / nmean)
            nc.scalar.mul(out=ex2[:], in_=bn2[:, 1:2], mul=1.0 / nmean)
            msq = p2.tile([128, 1], f32, tag="msq")
            nc.vector.tensor_tensor(out=msq[:], in0=mean[:], in1=mean[:],
                                    op=mybir.AluOpType.mult)
            var = p2.tile([128, 1], f32, tag="var")
            nc.vector.tensor_tensor(out=var[:], in0=ex2[:], in1=msq[:],
                                    op=mybir.AluOpType.subtract)
            std = p2.tile([128, 1], f32, tag="std")
            nc.scalar.activation(out=std[:], in_=var[:],
                                 func=mybir.ActivationFunctionType.Sqrt,
                                 bias=bneps_c, scale=1.0)
            rstd = p2.tile([128, 1], f32, tag="rstd")
            nc.vector.reciprocal(rstd[:], std[:])
            scl = p2.tile([128, 1], f32, tag="scl")
            nc.vector.tensor_tensor(out=scl[:], in0=gamma_c, in1=rstd[:],
                                    op=mybir.AluOpType.mult)
            mscl = p2.tile([128, 1], f32, tag="mscl")
            nc.vector.tensor_tensor(out=mscl[:], in0=mean[:], in1=scl[:],
                                    op=mybir.AluOpType.mult)
            shf = p2.tile([128, 1], f32, tag="shf")
            nc.vector.tensor_tensor(out=shf[:], in0=beta_c, in1=mscl[:],
                                    op=mybir.AluOpType.subtract)

            # final normalize+relu+store in a few fat strips so the ACT and
            # the out DMAs pipeline instead of ping-ponging
            OSTR = (NSLAB + 3) // 4
            for si in range(4):
                n0 = OSTR * si
                w = min(OSTR, NSLAB - n0)
                if w <= 0:
                    break
                ot = p2.tile([128, OSTR], bf16, tag="outt")
                nc.scalar.activation(out=ot[:, :w], in_=opre_sb[:, n0:n0 + w],
                                     func=mybir.ActivationFunctionType.Relu,
                                     bias=shf[:], scale=scl[:])
                eng = nc.sync if si % 2 == 0 else nc.scalar
                eng.dma_start(out=out_d[:, n0:n0 + w], in_=ot[:, :w])

    nc.compile()
    return nc


def last_graph(inputs):
    """Rebuild the exact graph kernel() ran, for cost-model timing."""
    schedule, _ = _preprocess(
        np.asarray(inputs["x"], np.float32), inputs["edge_index"],
        inputs["edge_attr"])
    return _build_graph(
        schedule, 1.0 + float(np.asarray(inputs["eps"]).reshape(-1)[0]))


def kernel(x, edge_index, edge_attr, edge_w, edge_b, w1, b1, w2, b2,
           res_w, res_b, eps, gamma, beta):
    global LAST_EXEC_NS, LAST_RESULTS
    x = np.asarray(x, dtype=np.float32)
    edge_w = np.asarray(edge_w, dtype=np.float32)
    edge_b = np.asarray(edge_b, dtype=np.float32)
    eps1 = 1.0 + float(np.asarray(eps).reshape(-1)[0])

    schedule, maps = _preprocess(x, edge_index, edge_attr)
    nc = _build_graph(schedule, eps1)

    x_bf16 = np.ascontiguousarray(x.astype(BF16))
    x2_bf16 = np.ascontiguousarray(
        np.concatenate([x_bf16, x_bf16], axis=1))
    consts = np.zeros((128, 389), dtype=np.float32)
    consts[:, 0:128] = np.broadcast_to(edge_w.reshape(1, C), (128, C))
    consts[:, 128:256] = np.broadcast_to(edge_b.reshape(1, C), (128, C))
    consts[:, 256:384] = np.eye(128, dtype=np.float32)
    consts[:, 384] = np.asarray(b1, dtype=np.float32)
    consts[:, 385] = np.asarray(b2, dtype=np.float32) + np.asarray(res_b, dtype=np.float32)
    consts[:, 386] = np.asarray(gamma, dtype=np.float32)
    consts[:, 387] = np.asarray(beta, dtype=np.float32)
    consts[:, 388] = BN_EPS
    iob = np.zeros((128, 257), dtype=np.float32)
    iob[:, 0:128] = np.broadcast_to(np.arange(128, dtype=np.float32), (128, 128))
    iob[:, 128] = 1.0
    iob[:, 129:257] = eps1 * np.eye(128, dtype=np.float32)
    iob = iob.astype(BF16)
    wts = np.concatenate([
        np.asarray(w1, dtype=np.float32),
        np.asarray(w2, dtype=np.float32),
        np.asarray(res_w, dtype=np.float32)], axis=1).astype(BF16)

    in_maps = []
    for i in range(NCORES):
        xt = np.ascontiguousarray(x[i * NSLAB:(i + 1) * NSLAB].T.astype(BF16))
        m = maps[i]
        in_maps.append({
            "x_bf16": x_bf16,
            "x2_bf16": x2_bf16,
            "pidx0": _padw(m["pidx0"]), "pidx1": _padw(m["pidx1"]),
            "sidx0": _padw(m["sidx0"]), "sidx1": _padw(m["sidx1"]),
            "dstrel": m["dstrel"],
            "cna": m["cna"],
            "x_t": xt,
            "consts_f32": consts,
            "iota_ones": iob,
            "wts": wts,
        })

    res = bass_utils.run_bass_kernel_spmd(
        nc, in_maps, core_ids=list(range(NCORES)), trace=TRACE)
    LAST_EXEC_NS = res.exec_time_ns
    LAST_RESULTS = res
    out = np.concatenate(
        [np.asarray(res.results[i]["out"]).T for i in range(NCORES)], axis=0)
    return out.astype(np.float32)


def _padw(a):
    """Pad an idx map to at least one call column (declared min width 8)."""
    if a.shape[1] == 0:
        return np.zeros((128, 8), np.int16)
    return a


# revision 55
# speedup vs baseline: 1.0090x; 1.0090x over previous
"""GINE layer (gather + edge-linear + scatter-mean + node MLP + BatchNorm + ReLU)
as a distributed Bass kernel on 8 TRN2 NeuronCores.

Sharding: edges are sharded by destination-node slab (N/8 nodes per core), so
each core's scatter-sums are complete locally.  Full x (bf16) is replicated in
each core's DRAM as the gather table; only BatchNorm statistics are
all-reduced.

Gather-descriptor pairing: random 256B-row gathers are descriptor-bound, so
edges that share a source node are paired into one 512B descriptor read from a
row-duplicated table x2 = [x|x].  A pair lands in two 128-slot slices of one
column; each slice is destination-block pure, so the scatter stays the plain
one-hot matmul.  Pairs are limited to block gap <= WG so at most WG+1
partial-sum blocks are open in PSUM at once.  The schedule (columns per
(half, b0, gap) class, single chunks per (half, block)) is sized by the max
over cores so the SPMD graph is shared; per-core slack is filled with
singleton edges, then sentinel pads.

Device pipeline per core:
  phase 1: dma_gather pair columns (512B descs) + single chunks (256B descs),
           build one-hot dst matrices on DVE (tensor_scalar is_equal vs iota,
           4x perf mode), TensorE matmul-scatter into per-block PSUM
           accumulators (window of WG+2 open blocks); per-block epilogue uses
           host-side cnt/attr-sum tables, PE-transposes into h_T.
  phase 2 (interleaved): node MLP on 512-node strips + BN stat accumulation.
  tail:    AllGather [sum, sumsq], normalize + relu, DMA out (bf16).
"""

import sys

sys.path.insert(0, "/opt/trn_rl_repo")

import numpy as np
import ml_dtypes

import concourse.bacc as bacc
import concourse.bass as bass
from concourse import mybir
from concourse.tile import TileContext
from concourse import bass_utils

BF16 = ml_dtypes.bfloat16

N = 50000
E = 1600000
C = 128
NCORES = 8
NSLAB = N // NCORES          # 6250 nodes per core
NBLK = (NSLAB + 127) // 128  # 49 dst blocks per core
HALF = N // 2                # 25000 (int16 gather index limit is 32767)
SENTINEL = 200.0             # never matches iota 0..127
BN_EPS = 1e-5
WG = 4                       # max block gap within a pair

# knobs (settable by test harness)
TRACE = False
LAST_EXEC_NS = None
LAST_RESULTS = None
COLLECTIVE = True
RACE_DETECT = True


def _preprocess(x, edge_index, edge_attr):
    """Host-side sharding + pairing. Returns (schedule, per-core maps)."""
    src = np.asarray(edge_index[0], dtype=np.int64)
    dst = np.asarray(edge_index[1], dtype=np.int64)
    attr = np.asarray(edge_attr[:, 0], dtype=np.float32)

    core_of = dst // NSLAB
    NCLS = 2 * NBLK * (WG + 1)          # class id = (h*NBLK + b0)*(WG+1) + g

    percore = []
    for i in range(NCORES):
        m = core_of == i
        s, d, a = src[m], dst[m] - i * NSLAB, attr[m]
        b = d // 128
        order = np.lexsort((b, s))
        s, d, b, a = s[order], d[order], b[order], a[order]
        Ei = len(s)
        # host-side scatter-mean denominators
        deg = np.bincount(d, minlength=NSLAB).astype(np.float32)
        asum = np.bincount(d, weights=a, minlength=NSLAB).astype(np.float32)
        # same-source adjacent pairing with block gap <= WG
        cand = np.zeros(Ei, bool)
        cand[:-1] = (s[1:] == s[:-1]) & ((b[1:] - b[:-1]) <= WG)
        startrun = cand & ~np.r_[False, cand[:-1]]
        runstart = np.maximum.accumulate(
            np.where(startrun, np.arange(Ei), 0))
        p0 = cand & ((np.arange(Ei) - runstart) % 2 == 0)
        p1 = np.r_[False, p0[:-1]]
        single = ~(p0 | p1)
        h = (s >= HALF).astype(np.int64)
        srel = s - h * HALF
        i1 = np.where(p0)[0] + 1
        cls = (h[p0] * NBLK + b[p0]) * (WG + 1) + (b[i1] - b[p0])
        pair_rec = (cls, srel[p0], (d[p0] % 128).astype(np.float32),
                    (d[i1] % 128).astype(np.float32))
        skey = h[single] * NBLK + b[single]
        sing_rec = (skey, srel[single], (d[single] % 128).astype(np.float32))
        percore.append((pair_rec, sing_rec, deg, asum))

    npairs = np.zeros((NCORES, NCLS), np.int64)
    for i in range(NCORES):
        npairs[i] = np.bincount(percore[i][0][0], minlength=NCLS)
    paircols = (npairs.max(axis=0) + 127) // 128

    # per-core leftover singles after filling pair-column slack
    nsing_left = np.zeros((NCORES, 2 * NBLK), np.int64)
    for i in range(NCORES):
        skey = percore[i][1][0]
        savail = np.bincount(skey, minlength=2 * NBLK)
        for c in np.where(paircols > 0)[0]:
            hb0, g = divmod(c, WG + 1)
            slack = paircols[c] * 128 - npairs[i][c]
            for k in (hb0, hb0 + g):
                take = min(slack, savail[k])
                savail[k] -= take
                slack -= take
        nsing_left[i] = savail
    singlechunks = (nsing_left.max(axis=0) + 127) // 128

    # shared emission order
    units = []          # ('P', h, b0, g) / ('S', h, b)
    for b in range(NBLK):
        for g in range(WG + 1):
            for h in (0, 1):
                c = (h * NBLK + b) * (WG + 1) + g
                if b + g < NBLK and paircols[c]:
                    units += [("P", h, b, g)] * int(paircols[c])
        for h in (0, 1):
            k = h * NBLK + b
            if singlechunks[k]:
                units += [("S", h, b)] * int(singlechunks[k])
    NSLICE = sum(2 if u[0] == "P" else 1 for u in units)
    pcols_h = [sum(1 for u in units if u[0] == "P" and u[1] == h)
               for h in (0, 1)]
    schk_h = [sum(1 for u in units if u[0] == "S" and u[1] == h)
              for h in (0, 1)]
    schedule = (tuple(units), tuple(pcols_h), tuple(schk_h), NSLICE)

    # per-core fill
    maps = []
    for i in range(NCORES):
        (pcls, psrel, pd0, pd1), (skey, ssrel, sdrel), deg, asum = percore[i]
        pair_by_cls = {}
        o = np.argsort(pcls, kind="stable")
        for c, lo, hi in zip(*_runs(pcls[o])):
            pair_by_cls[c] = (psrel[o[lo:hi]], pd0[o[lo:hi]], pd1[o[lo:hi]])
        sing_by_key = {}
        o = np.argsort(skey, kind="stable")
        for k, lo, hi in zip(*_runs(skey[o])):
            sing_by_key[k] = [ssrel[o[lo:hi]], sdrel[o[lo:hi]], 0]

        pidx = [np.zeros(pcols_h[h] * 128, np.int16) for h in (0, 1)]
        sidx = [np.zeros(schk_h[h] * 128, np.int16) for h in (0, 1)]
        dstrel = np.full((NSLICE, 128), SENTINEL, np.float32)
        pcur, scur, slice_i = [0, 0], [0, 0], 0
        ccur = {}
        for u in units:
            if u[0] == "P":
                h, b0, g = u[1], u[2], u[3]
                c = (h * NBLK + b0) * (WG + 1) + g
                pool = pair_by_cls.get(c)
                lo = ccur.get(c, 0)
                take = min(128, (len(pool[0]) if pool else 0) - lo)
                take = max(take, 0)
                ccur[c] = lo + take
                base = pcur[h] * 128
                if take:
                    pidx[h][base:base + take] = pool[0][lo:lo + take]
                    dstrel[slice_i, :take] = pool[1][lo:lo + take]
                    dstrel[slice_i + 1, :take] = pool[2][lo:lo + take]
                fill = 128 - take
                pos = take
                for sl, kk in ((0, h * NBLK + b0), (1, h * NBLK + b0 + g)):
                    if fill <= 0:
                        break
                    q = sing_by_key.get(kk)
                    if q is None:
                        continue
                    av = len(q[0]) - q[2]
                    t = min(fill, av)
                    if t > 0:
                        qlo = q[2]
                        pidx[h][base + pos:base + pos + t] = q[0][qlo:qlo + t]
                        dstrel[slice_i + sl, pos:pos + t] = q[1][qlo:qlo + t]
                        q[2] += t
                        pos += t
                        fill -= t
                pcur[h] += 1
                slice_i += 2
            else:
                h, b = u[1], u[2]
                q = sing_by_key.get(h * NBLK + b)
                av = (len(q[0]) - q[2]) if q else 0
                take = min(128, max(av, 0))
                base = scur[h] * 128
                if take:
                    qlo = q[2]
                    sidx[h][base:base + take] = q[0][qlo:qlo + take]
                    dstrel[slice_i, :take] = q[1][qlo:qlo + take]
                    q[2] += take
                scur[h] += 1
                slice_i += 1

        def wrap(v):
            # dma_gather idx layout: [128, n//16] wrapped in 16 partitions,
            # replicated for the 8 Q7 cores
            if len(v) == 0:
                return np.zeros((128, 0), np.int16)
            return np.ascontiguousarray(np.tile(v.reshape(-1, 16).T, (8, 1)))

        cna = np.zeros((128, 2 * NBLK), dtype=np.float32)
        padded = np.zeros(NBLK * 128, dtype=np.float32)
        padded[:NSLAB] = deg
        cna[:, 0:NBLK] = padded.reshape(NBLK, 128).T
        padded = np.zeros(NBLK * 128, dtype=np.float32)
        padded[:NSLAB] = asum
        cna[:, NBLK:2 * NBLK] = padded.reshape(NBLK, 128).T
        maps.append({
            "pidx0": wrap(pidx[0]), "pidx1": wrap(pidx[1]),
            "sidx0": wrap(sidx[0]), "sidx1": wrap(sidx[1]),
            "dstrel": np.ascontiguousarray(dstrel.T),
            "cna": cna,
        })
    return schedule, maps


def _runs(sorted_vals):
    """(values, run_starts, run_ends) for a sorted 1-D array."""
    if len(sorted_vals) == 0:
        return [], [], []
    change = np.r_[True, sorted_vals[1:] != sorted_vals[:-1]]
    starts = np.where(change)[0]
    ends = np.r_[starts[1:], len(sorted_vals)]
    return sorted_vals[starts], starts, ends


def _build_graph(schedule, eps1):
    """Build the SPMD Bass graph (same for all cores)."""
    f32 = mybir.dt.float32
    bf16 = mybir.dt.bfloat16
    units, pcols_h, schk_h, NSLICE = schedule
    G_CH = 8           # chunks/columns per gather call (1024-desc ring limit)
    NSTRIP = (NSLAB + 255) // 256

    nc = bacc.Bacc("TRN2", num_devices=NCORES, detect_race_conditions=RACE_DETECT)

    x_d = nc.declare_dram_parameter("x_bf16", [N, C], bf16, isOutput=False)
    x2_d = nc.declare_dram_parameter("x2_bf16", [N, 2 * C], bf16, isOutput=False)
    pidx_d = [nc.declare_dram_parameter(f"pidx{h}", [128, max(pcols_h[h], 1) * 8],
                                        mybir.dt.int16, isOutput=False)
              for h in (0, 1)]
    sidx_d = [nc.declare_dram_parameter(f"sidx{h}", [128, max(schk_h[h], 1) * 8],
                                        mybir.dt.int16, isOutput=False)
              for h in (0, 1)]
    dstrel_d = nc.declare_dram_parameter("dstrel", [128, NSLICE], f32, isOutput=False)
    cna_d = nc.declare_dram_parameter("cna", [128, 2 * NBLK], f32, isOutput=False)
    xt_d = nc.declare_dram_parameter("x_t", [128, NSLAB], bf16, isOutput=False)
    cf_d = nc.declare_dram_parameter("consts_f32", [128, 389], f32, isOutput=False)
    iob_d = nc.declare_dram_parameter("iota_ones", [128, 257], bf16, isOutput=False)
    wts_d = nc.declare_dram_parameter("wts", [128, 384], bf16, isOutput=False)
    out_d = nc.declare_dram_parameter("out", [128, NSLAB], bf16, isOutput=True)

    bn_in_d = nc.dram_tensor("bn_in", [128, 2], f32, kind="Internal")
    bn_out_d = nc.dram_tensor("bn_out", [NCORES * 128, 2], f32, kind="Internal", addr_space="Shared")

    xh = (x_d[0:HALF, :], x_d[HALF:N, :])
    x2h = (x2_d[0:HALF, :], x2_d[HALF:N, :])

    # per-block total touches (slices) for start/stop flags
    touches = [0] * NBLK
    for u in units:
        if u[0] == "P":
            touches[u[2]] += 1
            touches[u[2] + u[3]] += 1
        else:
            touches[u[2]] += 1

    # strip si fires when its last covering block's epilogue is done
    strip_of_block = {}
    for si in range(NSTRIP):
        last_blk = min((256 * si + min(256, NSLAB - 256 * si) - 1) // 128, NBLK - 1)
        strip_of_block.setdefault(last_blk, []).append(si)



    with TileContext(nc) as tc:
        with tc.tile_pool(name="persist", bufs=1) as pp, \
             tc.tile_pool(name="gpair", bufs=6) as gpp, \
             tc.tile_pool(name="gsing", bufs=6) as gsp, \
             tc.tile_pool(name="spool", bufs=6) as sp, \
             tc.tile_pool(name="eppool", bufs=2) as ep, \
             tc.tile_pool(name="p2pool", bufs=3) as p2, \
             tc.tile_pool(name="p1psum", bufs=WG + 2, space="PSUM") as p1p, \
             tc.tile_pool(name="tppsum", bufs=1, space="PSUM") as tpp, \
             tc.tile_pool(name="pmlp", bufs=1, space="PSUM") as pm:
            pidx_sb = [pp.tile([128, max(pcols_h[h], 1) * 8], mybir.dt.int16,
                               name=f"pidx_sb{h}") for h in (0, 1)]
            sidx_sb = [pp.tile([128, max(schk_h[h], 1) * 8], mybir.dt.int16,
                               name=f"sidx_sb{h}") for h in (0, 1)]
            dstrel_sb = pp.tile([128, NSLICE], f32)
            cna_sb = pp.tile([128, 2 * NBLK], f32)
            xt_sb = pp.tile([128, NSLAB], bf16)
            cf_sb = pp.tile([128, 389], f32)
            iob_sb = pp.tile([128, 257], bf16)
            wts_sb = pp.tile([128, 384], bf16)
            ht_sb = pp.tile([128, NSLAB], bf16)
            opre_sb = pp.tile([128, NSLAB], bf16)

            for h in (0, 1):
                w = max(pcols_h[h], 1) * 8
                nc.sync.dma_start(out=pidx_sb[h][:, 0:min(128, w)],
                                  in_=pidx_d[h][:, 0:min(128, w)])
                if w > 128:
                    nc.sync.dma_start(out=pidx_sb[h][:, 128:w],
                                      in_=pidx_d[h][:, 128:w])
                nc.sync.dma_start(out=sidx_sb[h][:], in_=sidx_d[h][:])
            # ordered by first use: iota gates the first one-hot build,
            # dstrel the first matmuls, cf the first epilogue
            nc.scalar.dma_start(out=iob_sb[:], in_=iob_d[:])
            nc.scalar.dma_start(out=dstrel_sb[:, 0:64], in_=dstrel_d[:, 0:64])
            nc.scalar.dma_start(out=dstrel_sb[:, 64:NSLICE],
                                in_=dstrel_d[:, 64:NSLICE])
            nc.scalar.dma_start(out=cf_sb[:], in_=cf_d[:])
            nc.scalar.dma_start(out=cna_sb[:], in_=cna_d[:])
            nc.scalar.dma_start(out=xt_sb[:], in_=xt_d[:])
            nc.scalar.dma_start(out=wts_sb[:], in_=wts_d[:])

            ew_b = cf_sb[:, 0:128]
            eb_b = cf_sb[:, 128:256]
            ident = cf_sb[:, 256:384]
            b1_c = cf_sb[:, 384:385]
            b2pr_c = cf_sb[:, 385:386]
            gamma_c = cf_sb[:, 386:387]
            beta_c = cf_sb[:, 387:388]
            bneps_c = cf_sb[:, 388:389]
            iota128 = iob_sb[:, 0:128]
            identeps_bf = iob_sb[:, 129:257]
            w1_s = wts_sb[:, 0:128]
            w2_s = wts_sb[:, 128:256]
            rw_s = wts_sb[:, 256:384]

            sum_cols = p2.tile([128, NSTRIP], f32, tag="sumc")
            sq_cols = p2.tile([128, NSTRIP], f32, tag="sqc")

            # touch Sqrt before the busy phase so the ACT function-table
            # load (1.3us) doesn't land on the BN tail's critical path
            warm = p2.tile([128, 1], f32, tag="warm")
            nc.scalar.activation(out=warm[:], in_=cf_sb[:, 388:389],
                                 func=mybir.ActivationFunctionType.Sqrt,
                                 bias=0.0, scale=1.0)

            def emit_strip(si):
                # 256-wide strips: hidden (pa) and output (po) partial sums
                # share one PSUM bank; their accumulation groups are strictly
                # sequential (relu fully consumes pa before mm2 starts)
                n0 = 256 * si
                w = min(256, NSLAB - n0)
                pt = pm.tile([128, 512], f32, tag="mlp", name=f"mlp{si}")
                pa = pt[:, 0:256]
                po = pt[:, 256:512]
                nc.tensor.matmul(out=pa[:, :w], lhsT=w1_s,
                                 rhs=ht_sb[:, n0:n0 + w], start=True, stop=True)
                hid = p2.tile([128, 256], bf16, tag="hid")
                nc.scalar.activation(out=hid[:, :w], in_=pa[:, :w],
                                     func=mybir.ActivationFunctionType.Relu,
                                     bias=b1_c, scale=1.0)
                nc.tensor.matmul(out=po[:, :w], lhsT=w2_s, rhs=hid[:, :w],
                                 start=True, stop=False)
                nc.tensor.matmul(out=po[:, :w], lhsT=rw_s,
                                 rhs=xt_sb[:, n0:n0 + w], start=False, stop=True)
                nc.scalar.activation(out=opre_sb[:, n0:n0 + w], in_=po[:, :w],
                                     func=mybir.ActivationFunctionType.Identity,
                                     bias=b2pr_c, scale=1.0,
                                     accum_out=sum_cols[:, si:si + 1])
                sq = p2.tile([128, 256], f32, tag="sq")
                nc.scalar.activation(out=sq[:, :w], in_=opre_sb[:, n0:n0 + w],
                                     func=mybir.ActivationFunctionType.Square,
                                     accum_out=sq_cols[:, si:si + 1])

            def psum_slice(b):
                if b not in open_psum:
                    open_psum[b] = p1p.tile([128, C], f32, tag="scat",
                                            name=f"scat{b}")
                return open_psum[b][:]

            def epilogue(b):
                ncol = NSLAB - b * 128 if b == NBLK - 1 else 128
                pt = psum_slice(b)
                blk = ep.tile([128, C], f32, tag="blk")
                nc.scalar.copy(out=blk[:], in_=pt[:])
                cnt_c = cna_sb[:, b:b + 1]
                asum_c = cna_sb[:, NBLK + b:NBLK + b + 1]
                cmax = ep.tile([128, 1], f32, tag="cmax")
                nc.vector.tensor_scalar_max(
                    out=cmax[:], in0=cnt_c, scalar1=1.0)
                recip = ep.tile([128, 1], f32, tag="recip")
                nc.vector.reciprocal(recip[:], cmax[:])
                t1 = ep.tile([128, C], f32, tag="ep1")
                nc.vector.scalar_tensor_tensor(
                    out=t1[:], in0=ew_b, scalar=asum_c,
                    in1=blk[:, 0:C],
                    op0=mybir.AluOpType.mult, op1=mybir.AluOpType.add)
                nc.vector.scalar_tensor_tensor(
                    out=t1[:], in0=eb_b, scalar=cnt_c,
                    in1=t1[:],
                    op0=mybir.AluOpType.mult, op1=mybir.AluOpType.add)
                aggr = ep.tile([128, C], f32, tag="aggr")
                nc.scalar.mul(out=aggr[:], in_=t1[:], mul=recip[:])
                ptt = tpp.tile([128, 128], f32, tag="tp", name=f"tp{b}")
                nc.tensor.matmul(out=ptt[:], lhsT=aggr[:], rhs=ident,
                                 is_transpose=True, start=True, stop=False)
                # accumulate (1+eps)*x_T via (eps1*I).T @ x_T on PE
                nc.tensor.matmul(out=ptt[:, 0:ncol], lhsT=identeps_bf,
                                 rhs=xt_sb[:, b * 128:b * 128 + ncol],
                                 start=False, stop=True)
                nc.scalar.copy(out=ht_sb[:, b * 128:b * 128 + ncol],
                               in_=ptt[:, 0:ncol])
                for si in strip_of_block.get(b, []):
                    emit_strip(si)

            # ---------------- phase 1 ----------------
            # 4 lazy gather streams: (pair, half) and (single, half)
            cursors = {("P", 0): 0, ("P", 1): 0, ("S", 0): 0, ("S", 1): 0}
            tiles = {k: [] for k in cursors}
            totals = {("P", 0): pcols_h[0], ("P", 1): pcols_h[1],
                      ("S", 0): schk_h[0], ("S", 1): schk_h[1]}

            def ensure(kind, h, col):
                # issue eagerly ~2 calls ahead of consumption so the Q7
                # descriptor generation never gates the SDMA stream
                key = (kind, h)
                while (not tiles[key] or tiles[key][-1][1] <= col
                       or (tiles[key][-1][1] < min(col + 2 * G_CH, totals[key])
                           and len(tiles[key]) < 3)):
                    lo = tiles[key][-1][1] if tiles[key] else 0
                    g = min(G_CH, totals[key] - lo)
                    if kind == "P":
                        gt = gpp.tile([128, g, 2 * C], bf16, tag="gpa")
                        nc.gpsimd.dma_gather(
                            out_ap=gt[:], in_ap=x2h[h],
                            idxs_ap=pidx_sb[h][:, lo * 8:(lo + g) * 8],
                            num_idxs=g * 128, num_idxs_reg=g * 128,
                            elem_size=2 * C)
                    else:
                        gt = gsp.tile([128, g, C], bf16, tag="gsi")
                        nc.gpsimd.dma_gather(
                            out_ap=gt[:], in_ap=xh[h],
                            idxs_ap=sidx_sb[h][:, lo * 8:(lo + g) * 8],
                            num_idxs=g * 128, num_idxs_reg=g * 128,
                            elem_size=C)
                    tiles[key].append((lo, lo + g, gt))
                    if len(tiles[key]) > 3:
                        tiles[key].pop(0)
                for lo, hi, gt in tiles[key]:
                    if lo <= col < hi:
                        return gt, col - lo
                raise AssertionError("gather tile evicted too early")

            open_psum = {}
            seen = [0] * NBLK
            stream_pos = {k: 0 for k in cursors}
            slice_i = 0

            def scat(bj, stile, rhs):
                seen[bj] += 1
                nc.tensor.matmul(out=psum_slice(bj), lhsT=stile, rhs=rhs,
                                 start=seen[bj] == 1,
                                 stop=seen[bj] == touches[bj])
                if seen[bj] == touches[bj]:
                    epilogue(bj)

            for u in units:
                if u[0] == "P":
                    h, b0, g = u[1], u[2], u[3]
                    col = stream_pos[("P", h)]
                    stream_pos[("P", h)] += 1
                    gt, lc = ensure("P", h, col)
                    for sl, bj in ((0, b0), (1, b0 + g)):
                        stile = sp.tile([128, 128], bf16, tag="sel")
                        nc.vector.tensor_scalar(
                            out=stile[:], in0=iota128,
                            scalar1=dstrel_sb[:, slice_i:slice_i + 1],
                            scalar2=None, op0=mybir.AluOpType.is_equal)
                        scat(bj, stile[:], gt[:, lc, sl * C:(sl + 1) * C])
                        slice_i += 1
                else:
                    h, b = u[1], u[2]
                    col = stream_pos[("S", h)]
                    stream_pos[("S", h)] += 1
                    gt, lc = ensure("S", h, col)
                    stile = sp.tile([128, 128], bf16, tag="sel")
                    nc.vector.tensor_scalar(
                        out=stile[:], in0=iota128,
                        scalar1=dstrel_sb[:, slice_i:slice_i + 1],
                        scalar2=None, op0=mybir.AluOpType.is_equal)
                    scat(b, stile[:], gt[:, lc, :])
                    slice_i += 1

            # blocks with zero touches (possible if a block got no edges)
            for b in range(NBLK):
                if touches[b] == 0:
                    zt = sp.tile([128, 128], bf16, tag="sel")
                    nc.vector.tensor_scalar(
                        out=zt[:], in0=iota128, scalar1=-1.0, scalar2=None,
                        op0=mybir.AluOpType.is_equal)
                    nc.tensor.matmul(out=psum_slice(b), lhsT=zt[:],
                                     rhs=xt_sb[:, 0:C], start=True, stop=True)
                    epilogue(b)

            # ---------------- BN tail ----------------
            ssum = p2.tile([128, 1], f32, tag="ssum")
            ssq = p2.tile([128, 1], f32, tag="ssq")
            nc.vector.tensor_reduce(out=ssum[:], in_=sum_cols[:],
                                    axis=mybir.AxisListType.X,
                                    op=mybir.AluOpType.add)
            nc.vector.tensor_reduce(out=ssq[:], in_=sq_cols[:],
                                    axis=mybir.AxisListType.X,
                                    op=mybir.AluOpType.add)
            bn_sb = p2.tile([128, 2], f32, tag="bn")
            nc.vector.tensor_copy(out=bn_sb[:, 0:1], in_=ssum[:])
            nc.vector.tensor_copy(out=bn_sb[:, 1:2], in_=ssq[:])
            nc.sync.dma_start(out=bn_in_d[:], in_=bn_sb[:])
            bn2 = p2.tile([128, 2], f32, tag="bn2")
            if COLLECTIVE:
                # AllGather + local reduce: priced well below AllReduce for
                # tiny payloads
                nc.gpsimd.collective_compute(
                    "AllGather", mybir.AluOpType.bypass,
                    replica_groups=[list(range(NCORES))],
                    ins=[bn_in_d[:].opt()], outs=[bn_out_d[:].opt()])
                bn8 = p2.tile([128, NCORES, 2], f32, tag="bn8")
                nc.sync.dma_start(
                    out=bn8[:],
                    in_=bass.AP(bn_out_d, 0, [(2, 128), (256, NCORES), (1, 2)]))
                bn8r = bn8[:]
                bn8v = bass.AP(bn8r.tensor, bn8r.offset,
                               [bn8r.ap[0], (1, 2), (2, NCORES)])
                nc.vector.tensor_reduce(out=bn2[:], in_=bn8v,
                                        axis=mybir.AxisListType.X,
                                        op=mybir.AluOpType.add)
                nmean = N
            else:
                nc.sync.dma_start(out=bn2[:], in_=bn_in_d[:])
                nmean = NSLAB

            mean = p2.tile([128, 1], f32, tag="mean")
            ex2 = p2.tile([128, 1], f32, tag="ex2")
            nc.scalar.mul(out=mean[:], in_=bn2[:, 0:1], mul=1.0 / nmean)
            nc.scalar.mul(out=ex2[:], in_=bn2[:, 1:2], mul=1.0 / nmean)
            msq = p2.tile([128, 1], f32, tag="msq")
            nc.vector.tensor_tensor(out=msq[:], in0=mean[:], in1=mean[:],
                                    op=mybir.AluOpType.mult)
            var = p2.tile([128, 1], f32, tag="var")
            nc.vector.tensor_tensor(out=var[:], in0=ex2[:], in1=msq[:],
                                    op=mybir.AluOpType.subtract)
            std = p2.tile([128, 1], f32, tag="std")
            nc.scalar.activation(out=std[:], in_=var[:],
                                 func=mybir.ActivationFunctionType.Sqrt,
                                 bias=bneps_c, scale=1.0)
            rstd = p2.tile([128, 1], f32, tag="rstd")
            nc.vector.reciprocal(rstd[:], std[:])
            scl = p2.tile([128, 1], f32, tag="scl")
            nc.vector.tensor_tensor(out=scl[:], in0=gamma_c, in1=rstd[:],
                                    op=mybir.AluOpType.mult)
            mscl = p2.tile([128, 1], f32, tag="mscl")
            nc.vector.tensor_tensor(out=mscl[:], in0=mean[:], in1=scl[:],
                                    op=mybir.AluOpType.mult)
            shf = p2.tile([128, 1], f32, tag="shf")
            nc.vector.tensor_tensor(out=shf[:], in0=beta_c, in1=mscl[:],
                                    op=mybir.AluOpType.subtract)

            # final normalize+relu+store in a few fat strips so the ACT and
            # the out DMAs pipeline instead of ping-ponging
            OSTR = (NSLAB + 3) // 4
            for si in range(4):
                n0 = OSTR * si
                w = min(OSTR, NSLAB - n0)
                if w <= 0:
                    break
                ot = p2.tile([128, OSTR], bf16, tag="outt")
                nc.scalar.activation(out=ot[:, :w], in_=opre_sb[:, n0:n0 + w],
                                     func=mybir.ActivationFunctionType.Relu,
                                     bias=shf[:], scale=scl[:])
                eng = nc.sync if si % 2 == 0 else nc.scalar
                eng.dma_start(out=out_d[:, n0:n0 + w], in_=ot[:, :w])

    nc.compile()
    return nc


def last_graph(inputs):
    """Rebuild the exact graph kernel() ran, for cost-model timing."""
    schedule, _ = _preprocess(
        np.asarray(inputs["x"], np.float32), inputs["edge_index"],
        inputs["edge_attr"])
    return _build_graph(
        schedule, 1.0 + float(np.asarray(inputs["eps"]).reshape(-1)[0]))


def kernel(x, edge_index, edge_attr, edge_w, edge_b, w1, b1, w2, b2,
           res_w, res_b, eps, gamma, beta):
    global LAST_EXEC_NS, LAST_RESULTS
    x = np.asarray(x, dtype=np.float32)
    edge_w = np.asarray(edge_w, dtype=np.float32)
    edge_b = np.asarray(edge_b, dtype=np.float32)
    eps1 = 1.0 + float(np.asarray(eps).reshape(-1)[0])

    schedule, maps = _preprocess(x, edge_index, edge_attr)
    nc = _build_graph(schedule, eps1)

    x_bf16 = np.ascontiguousarray(x.astype(BF16))
    x2_bf16 = np.ascontiguousarray(
        np.concatenate([x_bf16, x_bf16], axis=1))
    consts = np.zeros((128, 389), dtype=np.float32)
    consts[:, 0:128] = np.broadcast_to(edge_w.reshape(1, C), (128, C))
    consts[:, 128:256] = np.broadcast_to(edge_b.reshape(1, C), (128, C))
    consts[:, 256:384] = np.eye(128, dtype=np.float32)
    consts[:, 384] = np.asarray(b1, dtype=np.float32)
    consts[:, 385] = np.asarray(b2, dtype=np.float32) + np.asarray(res_b, dtype=np.float32)
    consts[:, 386] = np.asarray(gamma, dtype=np.float32)
    consts[:, 387] = np.asarray(beta, dtype=np.float32)
    consts[:, 388] = BN_EPS
    iob = np.zeros((128, 257), dtype=np.float32)
    iob[:, 0:128] = np.broadcast_to(np.arange(128, dtype=np.float32), (128, 128))
    iob[:, 128] = 1.0
    iob[:, 129:257] = eps1 * np.eye(128, dtype=np.float32)
    iob = iob.astype(BF16)
    wts = np.concatenate([
        np.asarray(w1, dtype=np.float32),
        np.asarray(w2, dtype=np.float32),
        np.asarray(res_w, dtype=np.float32)], axis=1).astype(BF16)

    in_maps = []
    for i in range(NCORES):
        xt = np.ascontiguousarray(x[i * NSLAB:(i + 1) * NSLAB].T.astype(BF16))
        m = maps[i]
        in_maps.append({
            "x_bf16": x_bf16,
            "x2_bf16": x2_bf16,
            "pidx0": _padw(m["pidx0"]), "pidx1": _padw(m["pidx1"]),
            "sidx0": _padw(m["sidx0"]), "sidx1": _padw(m["sidx1"]),
            "dstrel": m["dstrel"],
            "cna": m["cna"],
            "x_t": xt,
            "consts_f32": consts,
            "iota_ones": iob,
            "wts": wts,
        })

    res = bass_utils.run_bass_kernel_spmd(
        nc, in_maps, core_ids=list(range(NCORES)), trace=TRACE)
    LAST_EXEC_NS = res.exec_time_ns
    LAST_RESULTS = res
    out = np.concatenate(
        [np.asarray(res.results[i]["out"]).T for i in range(NCORES)], axis=0)
    return out.astype(np.float32)


def _padw(a):
    """Pad an idx map to at least one call column (declared min width 8)."""
    if a.shape[1] == 0:
        return np.zeros((128, 8), np.int16)
    return a
